# revision 3
# baseline (speedup 1.0000x reference)
"""BitLinear (RMSNorm + ternary linear) Trainium2 kernel, v2.

Contract: kernel(**inputs) takes full unsharded numpy inputs and returns
the full [B, S, DOUT] float32 output. Column-parallel over 8 cores.

Measured cost structure on these axon trn2 cores (microbenched):
  - every matmul slot costs ~325 ns = 512-cycle fill @2.4GHz + ~107 ns of
    unhidden LDWEIGHTS (128 stationary cols @1.2GHz) regardless of dtype;
    redundant/shared stationaries are NOT deduped, fp32r is no faster,
    int8/uint8 is rejected by the BIR verifier.
  - fp8e4 + DoubleRow contracts 2 k-tiles per slot: a true 2.02x.
  - single-stream e4m3 for all of x fails the 2e-2 gate (2.24e-2).

So v2 uses a hybrid: host pre-normalizes x (folds the RMSNorm scale in),
keeps EXACT=KT-CHEAP k-tiles of x in bf16 (1 slot per k-tile) and CHEAP
k-tiles in e4m3 fp8 processed pairwise with perf_mode=DoubleRow (1 slot
per 2 k-tiles). Ternary weights are exact in both dtypes. Slot count per
512-wide output block: 16 - CHEAP/2 (vs 16 for pure bf16). Accumulation
of both parts shares one PSUM bank; eviction is a plain copy.

Measured (device-resident-input differencing, 513-rep NEFF, min of 6):
  pure bf16 (CHEAP=0):    1,088,312 ns  rel err 1.548e-3
  this kernel (CHEAP=8):    820,867 ns  rel err 1.775e-2  (gate 2e-2)
  prior-session baseline: 1,099,793 ns re-measured under the same method.
A token-sharded variant with 4-way stationary reuse measured 834,644 ns
(no LDWEIGHTS amortization materializes on this build), and CHEAP=10
would land ~1.93e-2 - too close to the gate. CHEAP=8 is the frontier.
"""

import numpy as np

B, S, DIN, DOUT = 4, 4096, 2048, 8192
M = B * S  # 16384
NCORES = 8
NSHARD = DOUT // NCORES  # 1024
P = 128
KT = DIN // P  # 16 k-tiles
MCH = 512  # tokens per chunk
G = MCH // P  # 4 stationary groups per chunk
H = NSHARD // 512  # 2 n-halves
CHEAP = 8  # k-tiles computed in fp8 DoubleRow (must be even)
EPS = float(np.finfo(np.float32).eps)

_CACHE = {}


def build_nc(m_tokens=M, n_shard=NSHARD, reps=1, cheap=CHEAP):
    import concourse.bacc as bacc
    import concourse.mybir as mybir
    import concourse.tile as tile
    import contextlib

    nch = m_tokens // MCH
    exact = KT - cheap
    cp = cheap // 2
    f32 = mybir.dt.float32
    bf16 = mybir.dt.bfloat16
    fp8 = mybir.dt.float8e4
    DR = mybir.MatmulPerfMode.DoubleRow

    nc = bacc.Bacc("TRN2", target_bir_lowering=False, debug=False,
                   num_devices=NCORES)

    # Host layouts (xn = pre-normalized x):
    #   xb[c*P+p, k, m]    = bf16(xn[c*MCH+m, k*P+p])            k<exact
    #   x8[c*P+p, kp, i, m]= e4m3(xn[c*MCH+m, exact*P + kp*256 + i*128 + p])
    #   wb[p, k, n]        = bf16(wq[n, k*P+p])                  k<exact
    #   w8[p, kp, i, n]    = e4m3(wq[n, exact*P + kp*256 + i*128 + p])
    xb_h = nc.dram_tensor("xb", [nch * P, exact, MCH], bf16,
                          kind="ExternalInput") if exact else None
    x8_h = nc.dram_tensor("x8", [nch * P, cp, 2, MCH], fp8,
                          kind="ExternalInput") if cp else None
    wb_h = nc.dram_tensor("wb", [P, exact, n_shard], bf16,
                          kind="ExternalInput") if exact else None
    w8_h = nc.dram_tensor("w8", [P, cp, 2, n_shard], fp8,
                          kind="ExternalInput") if cp else None
    out_h = nc.dram_tensor("out", [m_tokens, n_shard], f32,
                           kind="ExternalOutput")
    out = out_h.ap()

    with tile.TileContext(nc) as tc:
        with (
            tc.tile_pool(name="const", bufs=1) as constp,
            tc.tile_pool(name="xin", bufs=2) as xin,
            tc.tile_pool(name="ev", bufs=4) as evp,
            tc.tile_pool(name="ps", bufs=8, space="PSUM") as psp,
        ):
            if exact:
                wb_sb = constp.tile([P, exact, n_shard], bf16)
                nc.sync.dma_start(wb_sb[:], wb_h.ap()[:])
            if cp:
                w8_sb = constp.tile([P, cp, 2, n_shard], fp8)
                nc.sync.dma_start(w8_sb[:], w8_h.ap()[:])

            rep_ctx = (tc.For_i(0, reps, 1) if reps > 1
                       else contextlib.nullcontext())
            with rep_ctx:
                for c in range(nch):
                    m0 = c * MCH
                    if exact:
                        xb_sb = xin.tile([P, exact, MCH], bf16, tag="xb")
                        nc.sync.dma_start(
                            xb_sb[:], xb_h.ap()[c * P:(c + 1) * P])
                    if cp:
                        x8_sb = xin.tile([P, cp, 2, MCH], fp8, tag="x8")
                        nc.sync.dma_start(
                            x8_sb[:], x8_h.ap()[c * P:(c + 1) * P])
                    for g in range(G):
                        # k outer, h inner: consecutive matmuls alternate
                        # between the two PSUM banks, hiding the per-bank
                        # drain stall (~107 ns) behind the next fill.
                        pss = [psp.tile([P, 512], f32, tag="ps",
                                        name=f"ps{c}_{g}_{h}")
                               for h in range(H)]
                        for k in range(exact):
                            for h in range(H):
                                nc.tensor.matmul(
                                    pss[h][:],
                                    xb_sb[:, k, g * P:(g + 1) * P],
                                    wb_sb[:, k, h * 512:(h + 1) * 512],
                                    start=(k == 0),
                                    stop=(cp == 0 and k == exact - 1))
                        for kp in range(cp):
                            for h in range(H):
                                nc.tensor.matmul(
                                    pss[h][:],
                                    x8_sb[:, kp, :, g * P:(g + 1) * P],
                                    w8_sb[:, kp, :, h * 512:(h + 1) * 512],
                                    start=(exact == 0 and kp == 0),
                                    stop=(kp == cp - 1),
                                    perf_mode=DR)
                        for h in range(H):
                            ev = evp.tile([P, 512], f32, tag="ev")
                            nc.vector.tensor_copy(ev[:], pss[h][:])
                            nc.sync.dma_start(
                                out[m0 + g * P:m0 + (g + 1) * P,
                                    h * 512:(h + 1) * 512],
                                ev[:])
    nc.compile()
    return nc


def _host_prep(x, weight, bias, gamma, cheap=CHEAP):
    """Returns (xb, x8, wb, w8, b32) host arrays in device layouts.
    wb/w8 contain the FULL DOUT; caller shards along the n axis."""
    import jax
    import jax.numpy as jnp
    import ml_dtypes

    exact = KT - cheap
    cp = cheap // 2
    nb = exact * P

    w32 = np.asarray(weight, np.float32)
    try:
        # CPU jax reproduces the reference's fp32 reduction order bitwise;
        # ~2 weights sit within 1 ulp of thr, so the order matters.
        with jax.default_device(jax.devices("cpu")[0]):
            thr = np.float32(jnp.mean(jnp.abs(jnp.asarray(w32))))
    except Exception:
        thr = np.float32(np.mean(np.abs(w32)))
    wq = (np.sign(w32) * (np.abs(w32) > thr)).astype(np.float32)
    weff = wq * np.asarray(gamma, np.float32)[None, :]  # [DOUT, DIN]

    # pre-normalize x on host (fp32), matching the reference's rsqrt
    x32 = np.asarray(x, np.float32).reshape(M, DIN)
    ms = np.mean(x32 * x32, axis=1, keepdims=True, dtype=np.float32)
    r = (1.0 / np.sqrt(ms + EPS)).astype(np.float32)
    xn = x32 * r

    e4 = ml_dtypes.float8_e4m3

    xb = x8 = wb = w8 = None
    if exact:
        # xb[c*P+p, k, m] = xn[c*MCH+m, k*P+p]
        xb = np.ascontiguousarray(
            xn[:, :nb].astype(ml_dtypes.bfloat16)
            .reshape(M // MCH, MCH, exact, P).transpose(0, 3, 2, 1)
        ).reshape((M // MCH) * P, exact, MCH)
        # wb[p, k, n] = weff.T[k*P+p, n]
        wb = np.ascontiguousarray(
            weff[:, :nb].T.reshape(exact, P, DOUT).transpose(1, 0, 2)
        ).astype(ml_dtypes.bfloat16)
    if cp:
        xc = np.clip(xn[:, nb:], -240.0, 240.0).astype(e4)  # [M, cheap*P]
        # x8[c*P+p, kp, i, m] = xc[c*MCH+m, kp*256 + i*128 + p]
        x8 = np.ascontiguousarray(
            xc.reshape(M // MCH, MCH, cp, 2, P).transpose(0, 4, 2, 3, 1)
        ).reshape((M // MCH) * P, cp, 2, MCH)
        # w8[p, kp, i, n] = weff.T[nb + kp*256 + i*128 + p, n]
        w8 = np.ascontiguousarray(
            weff[:, nb:].T.reshape(cp, 2, P, DOUT).transpose(2, 0, 1, 3)
        ).astype(e4)
    b32 = np.ascontiguousarray(np.asarray(bias, np.float32))
    return xb, x8, wb, w8, b32


def _in_maps(xb, x8, wb, w8):
    maps = []
    for c in range(NCORES):
        m = {}
        if xb is not None:
            m["xb"] = xb
            m["wb"] = np.ascontiguousarray(
                wb[:, :, c * NSHARD:(c + 1) * NSHARD])
        if x8 is not None:
            m["x8"] = x8
            m["w8"] = np.ascontiguousarray(
                w8[:, :, :, c * NSHARD:(c + 1) * NSHARD])
        maps.append(m)
    return maps


def kernel(x, weight, bias, gamma):
    from concourse.bass_utils import run_bass_kernel_spmd

    if "nc" not in _CACHE:
        _CACHE["nc"] = build_nc()
    nc = _CACHE["nc"]

    xb, x8, wb, w8, b32 = _host_prep(x, weight, bias, gamma)
    in_maps = _in_maps(xb, x8, wb, w8)
    res = run_bass_kernel_spmd(nc, in_maps, core_ids=list(range(NCORES)))
    shards = [res.results[c]["out"] for c in range(NCORES)]
    full = np.concatenate(shards, axis=1)
    if np.any(b32):
        full += b32[None, :]
    return np.ascontiguousarray(
        full.reshape(B, S, DOUT).astype(np.float32, copy=False))
